# revision 2
# baseline (speedup 1.0000x reference)
"""Trainium2 Bass kernel for a dense top-2 MoE (N=4096, D=1024, E=8, DFF=2048).

Sharding: data-parallel over tokens. Each of the 8 cores processes 512 tokens
through all 8 experts (the reference runs every expert on every token, so the
dense formulation is exact up to fp rounding). No collectives needed; the
gather is a concatenation of disjoint token shards.

Per-core device work:
  - gate logits in fp32 on the PE, exact top-2 + renormalized softmax weights
    computed with max / second-max masking on the vector engine
  - per expert e: hT = silu(w1_e^T-chunk @ xT) (bf16 matmul, fp32 PSUM
    accumulation, SiLU on the scalar engine), y = hT^T @ w2_e (bf16, fp32
    PSUM), then acc += w[token, e] * y fused on the vector engine
Host only shards/transposes/casts inputs and concatenates the output shards.
"""

import os
import sys

if "/opt/trn_rl_repo" not in sys.path:
    sys.path.insert(0, "/opt/trn_rl_repo")

import ml_dtypes
import numpy as np

import concourse.bass as bass
import concourse.mybir as mybir
import concourse.tile as tile
from concourse import bacc
from concourse.bass import ts
from concourse.bass_utils import run_bass_kernel_spmd

P = 128
N_TOK = 4096
D = 1024
E = 8
DFF = 2048
N_CORES = 8
TOK = N_TOK // N_CORES  # 512 tokens per core
KC_D = D // P           # 8 contraction chunks over D
KC_F = DFF // P         # 16 contraction chunks over DFF
NT = TOK // P           # 4 token tiles per core
FREE = 512              # matmul moving free dim (one PSUM bank of fp32)
ND = D // FREE          # 2 output halves

F32 = mybir.dt.float32
BF16 = mybir.dt.bfloat16
AF = mybir.ActivationFunctionType
ALU = mybir.AluOpType
AX = mybir.AxisListType

_CACHE = {}

# Results of the most recent run (for the local test harness to inspect).
LAST_RESULT = None


def _build():
    nc = bacc.Bacc(trn_type="TRN2", debug=False, num_devices=N_CORES)

    xT_d = nc.dram_tensor("xT", [D, TOK], F32, kind="ExternalInput").ap()
    gwT_d = nc.dram_tensor("gwT", [D, E], F32, kind="ExternalInput").ap()
    w1_d = nc.dram_tensor("w1", [E, D, DFF], BF16, kind="ExternalInput").ap()
    w2_d = nc.dram_tensor("w2", [E, DFF, D], BF16, kind="ExternalInput").ap()
    out_d = nc.dram_tensor("out", [TOK, D], F32, kind="ExternalOutput").ap()

    with tile.TileContext(nc) as tc:
        with (
            tc.tile_pool(name="const", bufs=1) as const_pool,
            tc.tile_pool(name="gate", bufs=2) as gate_pool,
            tc.tile_pool(name="w1p", bufs=2) as w1_pool,
            tc.tile_pool(name="w2p", bufs=2) as w2_pool,
            tc.tile_pool(name="htp", bufs=1) as ht_pool,
            tc.tile_pool(name="psh", bufs=2, space="PSUM") as psum_h,
            tc.tile_pool(name="psy", bufs=4, space="PSUM") as psum_y,
        ):
            # Resident inputs: transposed activations (fp32 for the gate,
            # bf16 for the expert matmuls) and transposed gate weights.
            xT = const_pool.tile([P, KC_D, TOK], F32, tag="xT")
            nc.sync.dma_start(xT[:], xT_d.rearrange("(kc p) t -> p kc t", p=P))
            xTb = const_pool.tile([P, KC_D, TOK], BF16, tag="xTb")
            nc.vector.tensor_copy(xTb[:], xT[:])
            gwT = const_pool.tile([P, KC_D, E], F32, tag="gwT")
            nc.sync.dma_start(gwT[:], gwT_d.rearrange("(kc p) e -> p kc e", p=P))

            # Gate: per 128-token tile, fp32 logits then normalized top-2
            # softmax weights W[token, expert] (zero outside the top-2).
            Wt = const_pool.tile([P, NT, E], F32, tag="Wt")
            for t in range(NT):
                pg_full = psum_h.tile([P, FREE], F32, tag="ph", name="pg")
                pg = pg_full[:, :E]
                for kc in range(KC_D):
                    nc.tensor.matmul(
                        pg[:],
                        xT[:, kc, ts(t, P)],
                        gwT[:, kc, :],
                        start=(kc == 0),
                        stop=(kc == KC_D - 1),
                    )
                L = gate_pool.tile([P, E], F32, tag="L")
                nc.vector.tensor_copy(L[:], pg[:])
                m1 = gate_pool.tile([P, 1], F32, tag="m1")
                nc.vector.reduce_max(m1[:], L[:], axis=AX.X)
                # mask out the max, then the max of the rest is the 2nd max
                L2 = gate_pool.tile([P, E], F32, tag="L2")
                ismax = gate_pool.tile([P, E], F32, tag="ismax")
                nc.vector.tensor_tensor(
                    ismax[:], L[:], m1.to_broadcast([P, E]), ALU.is_ge
                )
                nc.vector.scalar_tensor_tensor(
                    L2[:], ismax[:], -1.0e30, L[:], ALU.mult, ALU.add
                )
                m2 = gate_pool.tile([P, 1], F32, tag="m2")
                nc.vector.reduce_max(m2[:], L2[:], axis=AX.X)
                keep = gate_pool.tile([P, E], F32, tag="keep")
                nc.vector.tensor_tensor(
                    keep[:], L[:], m2.to_broadcast([P, E]), ALU.is_ge
                )
                negm1 = gate_pool.tile([P, 1], F32, tag="negm1")
                nc.vector.tensor_scalar_mul(negm1[:], m1[:], -1.0)
                expw = gate_pool.tile([P, E], F32, tag="expw")
                nc.scalar.activation(expw[:], L[:], AF.Exp, bias=negm1[:, 0:1])
                wun = gate_pool.tile([P, E], F32, tag="wun")
                nc.vector.tensor_mul(wun[:], expw[:], keep[:])
                den = gate_pool.tile([P, 1], F32, tag="den")
                nc.vector.reduce_sum(den[:], wun[:], axis=AX.X)
                inv = gate_pool.tile([P, 1], F32, tag="inv")
                nc.vector.reciprocal(inv[:], den[:])
                nc.vector.tensor_scalar_mul(Wt[:, t, :], wun[:], inv[:, 0:1])

            acc = const_pool.tile([P, NT, D], F32, tag="acc")
            nc.vector.memset(acc[:], 0.0)

            for e in range(E):
                w1t = w1_pool.tile([P, KC_D, DFF], BF16, tag="w1t")
                nc.sync.dma_start(
                    w1t[:], w1_d[e].rearrange("(kc p) f -> p kc f", p=P)
                )
                w2t = w2_pool.tile([P, KC_F, D], BF16, tag="w2t")
                nc.sync.dma_start(
                    w2t[:], w2_d[e].rearrange("(kc p) d -> p kc d", p=P)
                )

                # hT[f, tok] = silu(x @ w1_e)^T, bf16
                ht = ht_pool.tile([P, KC_F, TOK], BF16, tag="ht")
                for m in range(KC_F):
                    ph = psum_h.tile([P, FREE], F32, tag="ph")
                    for kc in range(KC_D):
                        nc.tensor.matmul(
                            ph[:],
                            w1t[:, kc, ts(m, P)],
                            xTb[:, kc, :],
                            start=(kc == 0),
                            stop=(kc == KC_D - 1),
                        )
                    nc.scalar.activation(ht[:, m, :], ph[:], AF.Silu)

                # y[tok, d] = hT^T @ w2_e, weighted-accumulated into acc
                for t in range(NT):
                    for dh in range(ND):
                        py = psum_y.tile([P, FREE], F32, tag="py")
                        for k in range(KC_F):
                            nc.tensor.matmul(
                                py[:],
                                ht[:, k, ts(t, P)],
                                w2t[:, k, ts(dh, FREE)],
                                start=(k == 0),
                                stop=(k == KC_F - 1),
                            )
                        nc.vector.scalar_tensor_tensor(
                            acc[:, t, ts(dh, FREE)],
                            py[:],
                            Wt[:, t, e : e + 1],
                            acc[:, t, ts(dh, FREE)],
                            ALU.mult,
                            ALU.add,
                        )

            nc.sync.dma_start(out_d.rearrange("(t p) d -> p t d", p=P), acc[:])

    nc.compile()
    return nc


def kernel(norm_data, gate_w, w1, w2):
    global LAST_RESULT
    if "nc" not in _CACHE:
        _CACHE["nc"] = _build()
    nc = _CACHE["nc"]

    x = np.ascontiguousarray(np.asarray(norm_data, dtype=np.float32))
    gwT = np.ascontiguousarray(np.asarray(gate_w, dtype=np.float32).T)
    w1b = np.ascontiguousarray(np.asarray(w1, dtype=np.float32)).astype(
        ml_dtypes.bfloat16
    )
    w2b = np.ascontiguousarray(np.asarray(w2, dtype=np.float32)).astype(
        ml_dtypes.bfloat16
    )

    in_maps = []
    for c in range(N_CORES):
        xT_c = np.ascontiguousarray(x[c * TOK : (c + 1) * TOK].T)
        in_maps.append({"xT": xT_c, "gwT": gwT, "w1": w1b, "w2": w2b})

    trace = os.environ.get("KERNEL_TRACE", "0") == "1"
    if trace:
        import antenv

        ext = os.environ.get("KERNEL_TRACE_HOOK_DIR", "/root/antenv_ext")
        if ext not in antenv.__path__:
            antenv.__path__.append(ext)
        from antenv.axon_hooks import set_axon_ntff_profile_hook
        from trn_agent_boot.trn_boot import _ntff_profile_via_ctypes

        set_axon_ntff_profile_hook(
            _ntff_profile_via_ctypes("/opt/axon/libaxon_pjrt.so")
        )

    res = run_bass_kernel_spmd(
        nc, in_maps, core_ids=list(range(N_CORES)), trace=trace
    )
    LAST_RESULT = res

    out = np.concatenate(
        [res.results[c]["out"] for c in range(N_CORES)], axis=0
    )
    return out


# revision 8
# speedup vs baseline: 1.4672x; 1.4672x over previous
"""Trainium2 Bass kernel for a dense top-2 MoE (N=4096, D=1024, E=8, DFF=2048).

Sharding: data-parallel over tokens. Each of the 8 cores processes 512 tokens
through all 8 experts (the reference runs every expert on every token, so the
dense formulation is exact up to fp rounding). No collectives needed; the
gather is a concatenation of disjoint token shards.

Per-core device work:
  - gate logits in fp32 on the PE, exact top-2 + renormalized softmax weights
    computed with max / second-max masking on the vector engine
  - per expert e: hT = silu(w1_e^T-chunk @ xT) (bf16 matmul, fp32 PSUM
    accumulation, SiLU on the scalar engine), y = hT^T @ w2_e (bf16, fp32
    PSUM), then acc += w[token, e] * y fused on the vector engine
Host only shards/transposes/casts inputs and concatenates the output shards.
"""

import os
import sys

if "/opt/trn_rl_repo" not in sys.path:
    sys.path.insert(0, "/opt/trn_rl_repo")

import ml_dtypes
import numpy as np

import concourse.bass as bass
import concourse.mybir as mybir
import concourse.tile as tile
from concourse import bacc
from concourse.bass import ts
from concourse.bass_utils import run_bass_kernel_spmd

P = 128
N_TOK = 4096
D = 1024
E = 8
DFF = 2048
N_CORES = 8
TOK = N_TOK // N_CORES  # 512 tokens per core
KC_D = D // P           # 8 contraction chunks over D
KC_F = DFF // P         # 16 contraction chunks over DFF
NT = TOK // P           # 4 token tiles per core
FREE = 512              # matmul moving free dim (one PSUM bank of fp32)
ND = D // FREE          # 2 output halves

F32 = mybir.dt.float32
F16 = mybir.dt.float16
AF = mybir.ActivationFunctionType
ALU = mybir.AluOpType
AX = mybir.AxisListType

_CACHE = {}

# Results of the most recent run (for the local test harness to inspect).
LAST_RESULT = None


def _build():
    nc = bacc.Bacc(trn_type="TRN2", debug=False, num_devices=N_CORES)

    xT_d = nc.dram_tensor("xT", [D, TOK], F32, kind="ExternalInput").ap()
    xTh_d = nc.dram_tensor("xTh", [D, TOK], F16, kind="ExternalInput").ap()
    gwT_d = nc.dram_tensor("gwT", [D, E], F32, kind="ExternalInput").ap()
    w1_d = nc.dram_tensor("w1", [E, D, DFF], F16, kind="ExternalInput").ap()
    w2_d = nc.dram_tensor("w2", [E, DFF, D], F16, kind="ExternalInput").ap()
    out_d = nc.dram_tensor("out", [TOK, D], F32, kind="ExternalOutput").ap()

    with tile.TileContext(nc) as tc:
        with (
            tc.tile_pool(name="const", bufs=1) as const_pool,
            tc.tile_pool(name="gate", bufs=2) as gate_pool,
            tc.tile_pool(name="w1p", bufs=2) as w1_pool,
            tc.tile_pool(name="w2p", bufs=2) as w2_pool,
            tc.tile_pool(name="htp", bufs=1) as ht_pool,
            tc.tile_pool(name="psh", bufs=2, space="PSUM") as psum_h,
            tc.tile_pool(name="psy", bufs=4, space="PSUM") as psum_y,
        ):
            # Resident inputs: transposed activations (fp32 for the gate,
            # bf16 for the expert matmuls) and transposed gate weights.
            xT = const_pool.tile([P, KC_D, TOK], F32, tag="xT")
            nc.sync.dma_start(xT[:], xT_d.rearrange("(kc p) t -> p kc t", p=P))
            xTb = const_pool.tile([P, KC_D, TOK], F16, tag="xTb")
            nc.sync.dma_start(xTb[:], xTh_d.rearrange("(kc p) t -> p kc t", p=P))
            gwT = const_pool.tile([P, KC_D, E], F32, tag="gwT")
            nc.sync.dma_start(gwT[:], gwT_d.rearrange("(kc p) e -> p kc e", p=P))

            # Gate: per 128-token tile, fp32 logits then normalized top-2
            # softmax weights W[token, expert] (zero outside the top-2).
            Wt = const_pool.tile([P, NT, E], F32, tag="Wt")
            for t in range(NT):
                pg_full = psum_h.tile([P, FREE], F32, tag="ph", name="pg")
                pg = pg_full[:, :E]
                for kc in range(KC_D):
                    nc.tensor.matmul(
                        pg[:],
                        xT[:, kc, ts(t, P)],
                        gwT[:, kc, :],
                        start=(kc == 0),
                        stop=(kc == KC_D - 1),
                    )
                L = gate_pool.tile([P, E], F32, tag="L")
                nc.vector.tensor_copy(L[:], pg[:])
                m1 = gate_pool.tile([P, 1], F32, tag="m1")
                nc.vector.reduce_max(m1[:], L[:], axis=AX.X)
                # mask out the max, then the max of the rest is the 2nd max
                L2 = gate_pool.tile([P, E], F32, tag="L2")
                ismax = gate_pool.tile([P, E], F32, tag="ismax")
                nc.vector.tensor_tensor(
                    ismax[:], L[:], m1.to_broadcast([P, E]), ALU.is_ge
                )
                nc.vector.scalar_tensor_tensor(
                    L2[:], ismax[:], -1.0e30, L[:], ALU.mult, ALU.add
                )
                m2 = gate_pool.tile([P, 1], F32, tag="m2")
                nc.vector.reduce_max(m2[:], L2[:], axis=AX.X)
                keep = gate_pool.tile([P, E], F32, tag="keep")
                nc.vector.tensor_tensor(
                    keep[:], L[:], m2.to_broadcast([P, E]), ALU.is_ge
                )
                negm1 = gate_pool.tile([P, 1], F32, tag="negm1")
                nc.vector.tensor_scalar_mul(negm1[:], m1[:], -1.0)
                expw = gate_pool.tile([P, E], F32, tag="expw")
                nc.scalar.activation(expw[:], L[:], AF.Exp, bias=negm1[:, 0:1])
                wun = gate_pool.tile([P, E], F32, tag="wun")
                nc.vector.tensor_mul(wun[:], expw[:], keep[:])
                den = gate_pool.tile([P, 1], F32, tag="den")
                nc.vector.reduce_sum(den[:], wun[:], axis=AX.X)
                inv = gate_pool.tile([P, 1], F32, tag="inv")
                nc.vector.reciprocal(inv[:], den[:])
                nc.vector.tensor_scalar_mul(Wt[:, t, :], wun[:], inv[:, 0:1])

            acc = const_pool.tile([P, NT, D], F32, tag="acc")
            nc.vector.memset(acc[:], 0.0)

            for e in range(E):
                w1t = w1_pool.tile([P, KC_D, DFF], F16, tag="w1t")
                nc.sync.dma_start(
                    w1t[:], w1_d[e].rearrange("(kc p) f -> p kc f", p=P)
                )
                w2t = w2_pool.tile([P, KC_F, D], F16, tag="w2t")
                nc.sync.dma_start(
                    w2t[:], w2_d[e].rearrange("(kc p) d -> p kc d", p=P)
                )

                # hT[f, tok] = silu(x @ w1_e)^T, fp16
                ht = ht_pool.tile([P, KC_F, TOK], F16, tag="ht")
                for m in range(KC_F):
                    ph = psum_h.tile([P, FREE], F32, tag="ph")
                    for kc in range(KC_D):
                        nc.tensor.matmul(
                            ph[:],
                            w1t[:, kc, ts(m, P)],
                            xTb[:, kc, :],
                            start=(kc == 0),
                            stop=(kc == KC_D - 1),
                        )
                    nc.scalar.activation(ht[:, m, :], ph[:], AF.Silu)

                # y[tok, d] = hT^T @ w2_e, weighted-accumulated into acc
                for t in range(NT):
                    for dh in range(ND):
                        py = psum_y.tile([P, FREE], F32, tag="py")
                        for k in range(KC_F):
                            nc.tensor.matmul(
                                py[:],
                                ht[:, k, ts(t, P)],
                                w2t[:, k, ts(dh, FREE)],
                                start=(k == 0),
                                stop=(k == KC_F - 1),
                            )
                        nc.vector.scalar_tensor_tensor(
                            acc[:, t, ts(dh, FREE)],
                            py[:],
                            Wt[:, t, e : e + 1],
                            acc[:, t, ts(dh, FREE)],
                            ALU.mult,
                            ALU.add,
                        )
                        if e == E - 1:
                            nc.sync.dma_start(
                                out_d.rearrange("(t p) d -> p t d", p=P)[
                                    :, t, ts(dh, FREE)
                                ],
                                acc[:, t, ts(dh, FREE)],
                            )

    nc.compile()
    return nc


def kernel(norm_data, gate_w, w1, w2):
    global LAST_RESULT
    if "nc" not in _CACHE:
        _CACHE["nc"] = _build()
    nc = _CACHE["nc"]

    x = np.ascontiguousarray(np.asarray(norm_data, dtype=np.float32))
    gwT = np.ascontiguousarray(np.asarray(gate_w, dtype=np.float32).T)
    w1b = np.ascontiguousarray(np.asarray(w1, dtype=np.float32)).astype(np.float16)
    w2b = np.ascontiguousarray(np.asarray(w2, dtype=np.float32)).astype(np.float16)

    in_maps = []
    for c in range(N_CORES):
        xT_c = np.ascontiguousarray(x[c * TOK : (c + 1) * TOK].T)
        in_maps.append(
            {
                "xT": xT_c,
                "xTh": xT_c.astype(np.float16),
                "gwT": gwT,
                "w1": w1b,
                "w2": w2b,
            }
        )

    trace = os.environ.get("KERNEL_TRACE", "0") == "1"
    if trace:
        import antenv

        ext = os.environ.get("KERNEL_TRACE_HOOK_DIR", "/root/antenv_ext")
        if ext not in antenv.__path__:
            antenv.__path__.append(ext)
        from antenv.axon_hooks import set_axon_ntff_profile_hook
        from trn_agent_boot.trn_boot import _ntff_profile_via_ctypes

        set_axon_ntff_profile_hook(
            _ntff_profile_via_ctypes("/opt/axon/libaxon_pjrt.so")
        )

    res = run_bass_kernel_spmd(
        nc, in_maps, core_ids=list(range(N_CORES)), trace=trace
    )
    LAST_RESULT = res

    out = np.concatenate(
        [res.results[c]["out"] for c in range(N_CORES)], axis=0
    )
    return out
